# revision 21
# baseline (speedup 1.0000x reference)
"""Chunked sliding-window attention (B=2, T=8192, H=16, Dh=128, W=256) on 8
Trainium2 NeuronCores.

Sharding: 8 cores = 2 (batch) x 4 (head groups of 4 heads). Each core computes
q/k/v projections for its 512-wide slice of the 2048 projection dims, RoPE,
chunked attention for its 4 heads, and a partial output projection over its
512 rows of Wo^T. The host sums the 4 partial outputs per batch element.

Device layouts (host-prepared):
  xt   [128, 16, T]   x^T tiles: xt[p, kt, t] = x[b, t, kt*128+p]        (bf16)
  wq/wk[128, 16, 512] (Wq_perm)^T slice, rope-split row permutation      (bf16)
  wv   [128, 16, 512] Wv^T slice (unpermuted)                            (bf16)
  wo   [128, 4, 2048] Wo^T rows for this core's 512 dims                 (bf16)
  ccat [128, T]       [cos; cos] rope table (freq idx on partitions)     (bf16)
  scat [128, T]       [-sin; sin]                                        (bf16)
  mask [128, 384]     [tril256 | tril128] causal masks, kv on partitions (bf16)

The rope row-permutation maps interleaved (re,im) pairs to split layout
(re block rows 0..63, im rows 64..127 per head); applied identically to q and
k it leaves scores invariant, and makes rope unit-stride on chip.

Attention is computed in transposed-score layout [kv, q]: softmax denominator
comes from an all-ones stationary matmul (broadcasts the per-q denominator
across all 128 partitions), masking is a 0/1 multiply after exp.

Schedule notes:
 - xt / rope tables for block N+1 are DMA-prefetched during block N.
 - The own-chunk upper kv tile (kv 128:256) only attends q-cols 128:256, so
   its score/denominator/AV matmuls stream 128 columns instead of 256.
 - Both score matmuls of a kv pair land in one PSUM bank and share one exp.
 - The deferred o-projection chains are interleaved two-per-head into the
   attention emission so the PE always has ready work while exps complete.
"""

import os

import numpy as np
import ml_dtypes

N_HEAD = 16
HEAD_DIM = 128
WINDOW = 256
THETA = 10000.0
B = 2
T = 8192
DM = 2048
KT = DM // 128      # 16 contraction tiles
HS = 4              # heads per core
DS = HS * HEAD_DIM  # 512 projection dims per core
BLK = 512           # tokens per pipeline block (2 chunks)
CH = WINDOW         # 256
MC = CH + 128       # combined own-chunk mask/e width (256 full + 128 skip-tile)
SCALE = float(HEAD_DIM) ** -0.5
# constant shift inside exp (softmax-invariant); keeps e <= ~exp(2.7) so the
# fp8e4 copy used by the DoubleRow denominator matmul cannot saturate (max 448)
EBIAS = -3.0

LAST_EXEC_NS = None
_NC = None

bf16 = ml_dtypes.bfloat16


def _build_nc(t_len=T):
    from contextlib import ExitStack

    import concourse.tile as tile
    from concourse import bacc, mybir

    fp32 = mybir.dt.float32
    b16 = mybir.dt.bfloat16
    f8 = mybir.dt.float8e4
    DR = mybir.MatmulPerfMode.DoubleRow

    nc = bacc.Bacc("TRN2", target_bir_lowering=False, debug=False)

    nb = t_len // BLK
    xt = nc.dram_tensor(
        "xt", [nb, 128, KT, BLK], b16, kind="ExternalInput"
    ).ap()
    wq = nc.dram_tensor("wq", [128, KT, DS], b16, kind="ExternalInput").ap()
    wk = nc.dram_tensor("wk", [128, KT, DS], b16, kind="ExternalInput").ap()
    wv = nc.dram_tensor("wv", [128, KT, DS], b16, kind="ExternalInput").ap()
    wo = nc.dram_tensor("wo", [128, HS, DM], b16, kind="ExternalInput").ap()
    ccat = nc.dram_tensor("ccat", [128, t_len], b16, kind="ExternalInput").ap()
    scat = nc.dram_tensor("scat", [128, t_len], b16, kind="ExternalInput").ap()
    mask = nc.dram_tensor("mask", [128, MC], b16, kind="ExternalInput").ap()
    y = nc.dram_tensor("y", [t_len, DM], b16, kind="ExternalOutput").ap()

    Exp = mybir.ActivationFunctionType.Exp

    with tile.TileContext(nc) as tc, ExitStack() as ctx:
        const = ctx.enter_context(tc.tile_pool(name="const", bufs=1))
        xt_p = ctx.enter_context(tc.tile_pool(name="xtp", bufs=2))
        raw_p = ctx.enter_context(tc.tile_pool(name="rawp", bufs=3))
        swp_p = ctx.enter_context(tc.tile_pool(name="swpp", bufs=3))
        tmp_p = ctx.enter_context(tc.tile_pool(name="tmpp", bufs=3))
        qr_p = ctx.enter_context(tc.tile_pool(name="qrp", bufs=8))
        kr_p = ctx.enter_context(tc.tile_pool(name="krp", bufs=10))
        v_p = ctx.enter_context(tc.tile_pool(name="vp", bufs=10))
        e_p = ctx.enter_context(tc.tile_pool(name="ep", bufs=3))
        e8_p = ctx.enter_context(tc.tile_pool(name="e8p", bufs=3))
        rc_p = ctx.enter_context(tc.tile_pool(name="rcp", bufs=3))
        ot_p = ctx.enter_context(tc.tile_pool(name="otp", bufs=24))
        y_p = ctx.enter_context(tc.tile_pool(name="yp", bufs=4))
        tab_p = ctx.enter_context(tc.tile_pool(name="tabp", bufs=3))
        ps_big = ctx.enter_context(tc.tile_pool(name="psbig", bufs=2, space="PSUM"))
        ps_st = ctx.enter_context(tc.tile_pool(name="psst", bufs=2, space="PSUM"))
        ps_st2 = ctx.enter_context(tc.tile_pool(name="psst2", bufs=2, space="PSUM"))
        ps_do = ctx.enter_context(tc.tile_pool(name="psdo", bufs=2, space="PSUM"))

        wq_sb = const.tile([128, KT, DS], b16)
        wk_sb = const.tile([128, KT, DS], b16)
        wv_sb = const.tile([128, KT, DS], b16)
        wo_sb = const.tile([128, HS, DM], b16)
        mask_sb = const.tile([128, MC], b16)
        ones_sb = const.tile([128, 128], b16)
        nc.vector.memset(ones_sb, 1.0)
        # fp8 all-ones stationary for the DoubleRow softmax-denominator matmul
        ones8_sb = const.tile([128, 2, 128], f8)
        nc.vector.memset(ones8_sb, 1.0)
        ebias_sb = const.tile([128, 1], fp32)
        nc.vector.memset(ebias_sb, EBIAS)

        tabs = {}

        def fetch_tables(blk):
            t0 = blk * BLK
            cc = tab_p.tile([128, BLK], b16, tag="cc")
            nc.sync.dma_start(cc, ccat[:, t0 : t0 + BLK])
            sc = tab_p.tile([128, BLK], b16, tag="sc")
            nc.sync.dma_start(sc, scat[:, t0 : t0 + BLK])
            tabs[blk] = (cc, sc)

        xts = {}

        def fetch_xt(blk):
            t = xt_p.tile([128, KT, BLK], b16, tag="xt")
            nc.sync.dma_start(t[:, 0:8, :], xt[blk][:, 0:8, :])
            nc.sync.dma_start(t[:, 8:KT, :], xt[blk][:, 8:KT, :])
            xts[blk] = t

        # Const + block-0/1 loads are split into small pieces emitted in the
        # order the first pipeline block consumes them; many small DMAs at the
        # head also engage more DMA engines while the issue pipe ramps up.
        xt0 = xt_p.tile([128, KT, BLK], b16, tag="xt")
        nc.sync.dma_start(wq_sb[:, 0:2, :], wq[:, 0:2, :])
        nc.sync.dma_start(xt0[:, 0:2, :], xt[0][:, 0:2, :])
        nc.sync.dma_start(wq_sb[:, 2:4, :], wq[:, 2:4, :])
        nc.sync.dma_start(xt0[:, 2:4, :], xt[0][:, 2:4, :])
        fetch_tables(0)
        nc.sync.dma_start(wq_sb[:, 4:8, :], wq[:, 4:8, :])
        nc.sync.dma_start(xt0[:, 4:8, :], xt[0][:, 4:8, :])
        nc.sync.dma_start(wq_sb[:, 8:KT, :], wq[:, 8:KT, :])
        nc.sync.dma_start(xt0[:, 8:KT, :], xt[0][:, 8:KT, :])
        xts[0] = xt0
        nc.sync.dma_start(wk_sb[:, 0:8, :], wk[:, 0:8, :])
        nc.sync.dma_start(wk_sb[:, 8:KT, :], wk[:, 8:KT, :])
        if nb > 1:
            fetch_tables(1)
            fetch_xt(1)
        nc.sync.dma_start(wv_sb[:, 0:8, :], wv[:, 0:8, :])
        nc.sync.dma_start(wv_sb[:, 8:KT, :], wv[:, 8:KT, :])
        nc.sync.dma_start(mask_sb, mask)
        for h in range(HS):
            nc.sync.dma_start(wo_sb[:, h, :], wo[:, h, :])

        prev_k = [None] * HS
        prev_v = [None, None]
        pend_ot = None

        def make_oproj_steps(ot_map, base_t0, fine, alt=False):
            state = {}
            steps = []

            def chain(tt, ct):
                def f():
                    if ct == 0:
                        state[tt] = y_p.tile([128, DM], b16, tag="y", name="ysb")
                    ysb = state[tt]
                    ci, sub = tt // 2, tt % 2
                    yps = ps_big.tile([128, 512], fp32, tag="psbig")
                    for h in range(HS):
                        nc.tensor.matmul(
                            yps,
                            lhsT=ot_map[(h, ci)][:, sub * 128 : (sub + 1) * 128],
                            rhs=wo_sb[:, h, ct * 512 : (ct + 1) * 512],
                            start=(h == 0),
                            stop=(h == HS - 1),
                        )
                    if ct % 2 == 0:
                        nc.scalar.copy(ysb[:, ct * 512 : (ct + 1) * 512], yps)
                    else:
                        nc.vector.tensor_copy(
                            out=ysb[:, ct * 512 : (ct + 1) * 512], in_=yps
                        )
                    r0 = base_t0 + tt * 128
                    if fine:
                        eng = nc.scalar if (alt and ct % 2) else nc.sync
                        eng.dma_start(
                            y[r0 : r0 + 128, ct * 512 : (ct + 1) * 512],
                            ysb[:, ct * 512 : (ct + 1) * 512],
                        )
                    elif ct == 3:
                        nc.sync.dma_start(y[r0 : r0 + 128, :], ysb)

                return f

            for tt in range(4):
                for ct in range(4):
                    steps.append(chain(tt, ct))
            return steps

        for blk in range(nb):
            t0 = blk * BLK
            xt_sb = xts.pop(blk)
            c_sl, s_sl = tabs.pop(blk)
            if blk >= 1 and blk + 1 < nb:
                fetch_xt(blk + 1)
                fetch_tables(blk + 1)

            cur_q = []
            cur_k = []
            for h in range(HS):
                for w_sb, dst in ((wq_sb, cur_q), (wk_sb, cur_k)):
                    ps = ps_big.tile([128, BLK], fp32, tag="psbig")
                    for k in range(KT):
                        nc.tensor.matmul(
                            ps,
                            lhsT=w_sb[:, k, h * 128 : (h + 1) * 128],
                            rhs=xt_sb[:, k, :],
                            start=(k == 0),
                            stop=(k == KT - 1),
                        )
                    raw = raw_p.tile([128, BLK], b16, tag="raw")
                    nc.scalar.copy(raw, ps)
                    # swap the (re, im) halves via SBUF->SBUF DMA (DVE lanes
                    # cannot cross partitions)
                    swp = swp_p.tile([128, BLK], b16, tag="swp")
                    nc.sync.dma_start(swp[0:64, :], raw[64:128, :])
                    nc.sync.dma_start(swp[64:128, :], raw[0:64, :])
                    t1 = tmp_p.tile([128, BLK], b16, tag="t1")
                    nc.vector.tensor_mul(t1, raw, c_sl)
                    t2 = tmp_p.tile([128, BLK], b16, tag="t2")
                    nc.vector.tensor_mul(t2, swp, s_sl)
                    if dst is cur_q:
                        rot = qr_p.tile([128, BLK], b16, tag="qr")
                    else:
                        rot = kr_p.tile([128, BLK], b16, tag="kr")
                    nc.vector.tensor_add(rot, t1, t2)
                    dst.append(rot)

            cur_v = []
            for tt in range(4):
                ps = ps_big.tile([128, BLK], fp32, tag="psbig")
                for k in range(KT):
                    nc.tensor.matmul(
                        ps,
                        lhsT=xt_sb[:, k, tt * 128 : (tt + 1) * 128],
                        rhs=wv_sb[:, k, :],
                        start=(k == 0),
                        stop=(k == KT - 1),
                    )
                vt = v_p.tile([128, DS], b16, tag="v")
                nc.vector.tensor_copy(out=vt, in_=ps)
                cur_v.append(vt)

            ot_tiles = {}

            def emit_scores(ci, h):
                c = 2 * blk + ci
                qoff = ci * CH
                q_sl = cur_q[h][:, qoff : qoff + CH]
                e01 = None
                e8 = None
                if c > 0:
                    st01 = ps_st.tile([128, 2, CH], fp32, tag="st")
                    for jj in range(2):
                        if ci == 1:
                            ksrc = cur_k[h][:, jj * 128 : (jj + 1) * 128]
                        else:
                            ksrc = prev_k[h][:, CH + jj * 128 : CH + (jj + 1) * 128]
                        nc.tensor.matmul(
                            st01[:, jj, :],
                            lhsT=ksrc, rhs=q_sl,
                            start=(jj == 0), stop=(jj == 1),
                        )
                    e01 = e_p.tile([128, 2, CH], b16, tag="e")
                    nc.scalar.activation(e01, st01, Exp, scale=SCALE, bias=ebias_sb)
                    # fp8 copy feeds the DoubleRow denominator matmul (2x rate)
                    e8 = e8_p.tile([128, 2, CH], f8, tag="e8")
                    nc.vector.tensor_copy(out=e8, in_=e01)
                # own-chunk pair: kv 0:128 over all 256 q-cols, kv 128:256
                # only over q-cols 128:256 (the rest is fully masked)
                st23 = ps_st2.tile([128, 512], fp32, tag="st2")
                nc.tensor.matmul(
                    st23[:, 0:CH],
                    lhsT=cur_k[h][:, qoff : qoff + 128], rhs=q_sl,
                    start=True, stop=False,
                )
                nc.tensor.matmul(
                    st23[:, CH:MC],
                    lhsT=cur_k[h][:, qoff + 128 : qoff + CH],
                    rhs=cur_q[h][:, qoff + 128 : qoff + CH],
                    start=False, stop=True,
                )
                e23 = e_p.tile([128, MC], b16, tag="e2")
                nc.scalar.activation(e23, st23[:, 0:MC], Exp, scale=SCALE, bias=ebias_sb)
                nc.vector.tensor_mul(e23, e23, mask_sb)
                return (ci, h, e01, e8, e23)

            def emit_finish(g):
                ci, h, e01, e8, e23 = g
                do = ps_do.tile([128, 512], fp32, tag="do")
                dn = do[:, 0:CH]
                ou = do[:, CH : 2 * CH]
                hs = slice(h * 128, (h + 1) * 128)
                if e01 is not None:
                    nc.tensor.matmul(dn, lhsT=ones8_sb, rhs=e8,
                                     start=True, stop=False, perf_mode=DR)
                    nc.tensor.matmul(dn, lhsT=ones_sb, rhs=e23[:, 0:CH],
                                     start=False, stop=False)
                    nc.tensor.matmul(dn[:, 128:CH], lhsT=ones_sb, rhs=e23[:, CH:MC],
                                     start=False, stop=True)
                else:
                    nc.tensor.matmul(dn, lhsT=ones_sb, rhs=e23[:, 0:CH],
                                     start=True, stop=False)
                    nc.tensor.matmul(dn[:, 128:CH], lhsT=ones_sb, rhs=e23[:, CH:MC],
                                     start=False, stop=True)
                v2 = cur_v[2 * ci]
                v3 = cur_v[2 * ci + 1]
                if e01 is not None:
                    v0 = cur_v[0] if ci == 1 else prev_v[0]
                    v1 = cur_v[1] if ci == 1 else prev_v[1]
                    nc.tensor.matmul(ou, lhsT=v0[:, hs], rhs=e01[:, 0, :],
                                     start=True, stop=False)
                    nc.tensor.matmul(ou, lhsT=v1[:, hs], rhs=e01[:, 1, :],
                                     start=False, stop=False)
                    nc.tensor.matmul(ou, lhsT=v2[:, hs], rhs=e23[:, 0:CH],
                                     start=False, stop=False)
                    nc.tensor.matmul(ou[:, 128:CH], lhsT=v3[:, hs], rhs=e23[:, CH:MC],
                                     start=False, stop=True)
                else:
                    nc.tensor.matmul(ou, lhsT=v2[:, hs], rhs=e23[:, 0:CH],
                                     start=True, stop=False)
                    nc.tensor.matmul(ou[:, 128:CH], lhsT=v3[:, hs], rhs=e23[:, CH:MC],
                                     start=False, stop=True)
                rc = rc_p.tile([128, CH], fp32, tag="rc")
                nc.vector.reciprocal_approx_fast(out=rc, in_=dn)
                ot = ot_p.tile([128, CH], b16, tag="ot")
                nc.vector.tensor_mul(ot, ou, rc)
                ot_tiles[(h, ci)] = ot

            # o-projection of the previous block is interleaved two chains
            # per attention group so the PE never idles on exp latency
            osteps = (
                make_oproj_steps(pend_ot[0], pend_ot[1], fine=(blk == nb - 1))
                if pend_ot is not None else []
            )
            # On the last block, its own o-projection chains for ci=0 (which
            # only need the first four finishes) are also interleaved so the
            # final drain is short.
            own_steps = (
                make_oproj_steps(ot_tiles, t0, fine=True, alt=True)
                if blk == nb - 1 else None
            )
            own_sched = {5: (0, 3), 6: (3, 6), 7: (6, 8)}
            si = 0
            prev_g = None
            for gi in range(2 * HS):
                ci, h = gi // HS, gi % HS
                g = emit_scores(ci, h)
                for _ in range(2):
                    if si < len(osteps):
                        osteps[si]()
                        si += 1
                if prev_g is not None:
                    emit_finish(prev_g)
                prev_g = g
                if own_steps is not None and gi in own_sched:
                    lo, hi = own_sched[gi]
                    for step in own_steps[lo:hi]:
                        step()
            emit_finish(prev_g)
            while si < len(osteps):
                osteps[si]()
                si += 1

            pend_ot = (ot_tiles, t0)
            if blk == nb - 1:
                for step in own_steps[8:]:
                    step()
                pend_ot = None

            prev_k = cur_k
            prev_v = cur_v[2:4]

    nc.compile()
    return nc


def _rope_perm():
    perm = np.empty(DM, np.int64)
    for h in range(N_HEAD):
        base = h * HEAD_DIM
        perm[base : base + 64] = base + 2 * np.arange(64)
        perm[base + 64 : base + 128] = base + 2 * np.arange(64) + 1
    return perm


def _prep_inputs(x, Wq, Wk, Wv, Wo, t_len=T):
    """Build per-core in_maps. Cores 0-3: batch 0, head groups 0-3; 4-7: batch 1."""
    x = np.asarray(x, dtype=np.float32)
    Wq = np.asarray(Wq, dtype=np.float32)
    Wk = np.asarray(Wk, dtype=np.float32)
    Wv = np.asarray(Wv, dtype=np.float32)
    Wo = np.asarray(Wo, dtype=np.float32)
    nb_b = x.shape[0]

    perm = _rope_perm()
    wqT = np.ascontiguousarray(Wq[perm].T).astype(bf16)  # [K, dout_perm]
    wkT = np.ascontiguousarray(Wk[perm].T).astype(bf16)
    wvT = np.ascontiguousarray(Wv.T).astype(bf16)
    woT = np.ascontiguousarray(Wo.T).astype(bf16)        # [d, c]

    # xt[blk, p, kt, t_in_blk] = x[b, blk*BLK + t, kt*128+p] — block-major so
    # each block's slab is one fully-contiguous DMA read per partition
    nblk = t_len // BLK
    xts = []
    for b in range(nb_b):
        xT = x[b].T.reshape(KT, 128, nblk, BLK)
        xts.append(np.ascontiguousarray(xT.transpose(2, 1, 0, 3)).astype(bf16))

    wq_s, wk_s, wv_s, wo_s = [], [], [], []
    for hg in range(4):
        sl = slice(hg * DS, (hg + 1) * DS)
        wq_s.append(np.ascontiguousarray(
            wqT[:, sl].reshape(KT, 128, DS).transpose(1, 0, 2)).astype(bf16))
        wk_s.append(np.ascontiguousarray(
            wkT[:, sl].reshape(KT, 128, DS).transpose(1, 0, 2)).astype(bf16))
        wv_s.append(np.ascontiguousarray(
            wvT[:, sl].reshape(KT, 128, DS).transpose(1, 0, 2)).astype(bf16))
        wo_s.append(np.ascontiguousarray(
            woT[sl].reshape(HS, 128, DM).transpose(1, 0, 2)).astype(bf16))

    inv = 1.0 / THETA ** (np.arange(0, HEAD_DIM, 2, dtype=np.float32) / HEAD_DIM)
    fr = np.outer(inv, np.arange(t_len, dtype=np.float32))  # [64, T]
    cosT = np.cos(fr).astype(np.float32)
    sinT = np.sin(fr).astype(np.float32)
    ccat = np.concatenate([cosT, cosT], axis=0).astype(bf16)   # [128, T]
    scat = np.concatenate([-sinT, sinT], axis=0).astype(bf16)  # [128, T]

    r = np.arange(128)[:, None]
    qc = np.arange(CH)[None, :]
    cc = np.arange(128)[None, :]
    # [tril over all 256 q-cols for kv 0:128 | tril over q-cols 128:256 for
    # kv 128:256]
    mask = np.concatenate([(r <= qc), (r <= cc)], axis=1).astype(bf16)  # [128,384]

    in_maps = []
    for core in range(8):
        b, hg = core // 4, core % 4
        in_maps.append({
            "xt": xts[b], "wq": wq_s[hg], "wk": wk_s[hg], "wv": wv_s[hg],
            "wo": wo_s[hg], "ccat": ccat, "scat": scat, "mask": mask,
        })
    return in_maps


def kernel(x, Wq, Wk, Wv, Wo):
    global _NC, LAST_EXEC_NS
    from concourse.bass_utils import run_bass_kernel_spmd

    profile = bool(os.environ.get("KERNEL_PROFILE"))
    if profile:
        try:
            import hook_util
            hook_util.install()
            hook_util.patch_upload()
        except ImportError:
            profile = False

    in_maps = _prep_inputs(x, Wq, Wk, Wv, Wo)
    if _NC is None:
        _NC = _build_nc()

    kwargs = {}
    if profile:
        kwargs["tmpdir"] = os.environ.get("KERNEL_TRACE_DIR") or None
    res = run_bass_kernel_spmd(
        _NC, in_maps, core_ids=list(range(8)), trace=profile, **kwargs
    )
    LAST_EXEC_NS = res.exec_time_ns

    out = np.zeros((B, T, DM), dtype=np.float32)
    for core in range(8):
        out[core // 4] += res.results[core]["y"].astype(np.float32)
    return out
